# revision 1
# baseline (speedup 1.0000x reference)
"""AreaSelfAttention Trainium2 kernel (8 NeuronCores, pure data parallel).

Reference computation (per full input):
  pad x [4,256,252,252] -> [4,256,256,256]; 1x1 convs q,k (256->32), v (256->256);
  8x8 windows; attn = softmax(q^T k) over j; out = v @ attn^T; unwindow;
  final = gamma * out + x  (crop back to 252x252).

Design:
  - Shard over (batch, wrow-half): each core gets 16 "wrows" of 2048
    window-major pixels (32 windows). x ships ONCE per core as fp8 e4m3 in
    [128, 2(c-half), pix] layout; Wq/Wk/Wv ship fp8 (unscaled - fp8-safe
    magnitudes). Device output is UNNORMALIZED PV plus a rowsum carrier
    column, bf16 [wrow, 128, 16, 257].
  - Host finishes: out = x + gamma*bv + gamma*(oT'/rowsum) in f32. This uses
    softmax affinity (attn(v+bv) = attn(v)+bv) so no bias/residual data ever
    ships to the device, and keeps the device free of normalization work.
  - K-bias is dropped on device ((q+bq).(k+bk) - (q+bq).k is constant per
    query -> softmax-invariant); Q-bias rides the ACT bias operand during the
    qk psum evac. No rank-1 bias matmuls.
  - Per wrow: qk conv (fp8, col-packed [q;k] psum halves) -> DVE
    tensor_scalar_add evac (+Q bias; DVE is idle at wrow start, so the
    gather chain is not queued behind ACT's exp backlog) -> merged
    SBUF->SBUF DMA gathers of q0/k0 to partition base 0 ->
    window-level sT matmuls ([32,64] stationaries, two column bands run in
    distinct PE sub-arrays; only diagonal blocks written) -> ACT Exp evac ->
    GPSIMD zeroes the cross-window blocks of eT -> pair-level PV (K=128,
    N=257 with carrier; eT off-diag zeros kill cross terms) -> plain
    ACT/DVE copy evacs (bf16) -> one out DMA per wrow.
  - vT conv: x-block-stationary fp8 matmuls -> vt[128,16,257] with col 256
    memset to 1.0 once per wrow (rowsum carrier through PV).
  - Software pipeline: x prefetched 2 wrows ahead; prev wrow's PV emitted
    between this wrow's vT conv and sT so the PE stays busy across the
    qk-evac -> gather -> sT dependency; psum: 2 score/qk banks + 2 vT banks
    + 2x2 PV banks = 8.
"""

from contextlib import ExitStack

import numpy as np
import ml_dtypes

import bass_rust as br
import concourse.bass as bass
import concourse.tile as tile
from concourse import mybir
from concourse.bass_utils import run_bass_kernel_spmd

FP32 = mybir.dt.float32
BF16 = mybir.dt.bfloat16
F8 = mybir.dt.float8e4
AF = mybir.ActivationFunctionType

B, C, H, W = 4, 256, 252, 252
A = 8
PH = PW = 256
NH = NW = 32
CR = 32
NCORES = 8
G = 16          # wrows per core
PIX = 2048      # pixels per wrow (32 windows * 64)


def _split_wide_waits(nc, max_waits=1):
    """walrus on this toolchain rejects >1 sync wait per instruction; move
    excess waits onto preceding same-engine NoOps (equivalent semantics)."""
    n = 0
    for fn in nc.m.functions:
        for bb in fn.blocks:
            insts = list(bb.instructions)
            new, changed = [], False
            for inst in insts:
                si = inst.sync_info
                waits = list(si.on_wait) if si is not None else []
                if len(waits) > max_waits:
                    changed = True
                    chunks = [waits[i:i + max_waits]
                              for i in range(0, len(waits), max_waits)]
                    for ch in chunks[:-1]:
                        nop = br.InstNoOp(name=f"I-wsplit-{n}", ins=[], outs=[])
                        n += 1
                        nop.engine = inst.engine
                        nop.sync_info = br.SyncInfo(on_wait=ch, on_update=[])
                        new.append(nop)
                    inst.sync_info = br.SyncInfo(
                        on_wait=chunks[-1], on_update=list(si.on_update))
                new.append(inst)
            if changed:
                bb.instructions = new
    return n


def build_nc():
    nc = bass.Bass()
    x_d = nc.declare_dram_parameter("x", [G, 128, 2, PIX], F8, isOutput=False)
    wqk_d = nc.declare_dram_parameter("wqk", [128, 2, 64], F8, isOutput=False)
    wvt_d = nc.declare_dram_parameter("wvt", [2, 128, 256], F8,
                                      isOutput=False)
    bqk_d = nc.declare_dram_parameter("bqk", [128, 1], FP32, isOutput=False)
    out_d = nc.declare_dram_parameter("out", [G, 128, 16, C + 1], BF16,
                                      isOutput=True)

    with tile.TileContext(nc) as tc, ExitStack() as ctx:
        consts = ctx.enter_context(tc.tile_pool(name="consts", bufs=1))
        xbp = ctx.enter_context(tc.tile_pool(name="xbp", bufs=3))
        qk2p = ctx.enter_context(tc.tile_pool(name="qk2p", bufs=2))
        qkg = ctx.enter_context(tc.tile_pool(name="qkg", bufs=2))
        etp = ctx.enter_context(tc.tile_pool(name="etp", bufs=2))
        vtp = ctx.enter_context(tc.tile_pool(name="vtp", bufs=2))
        otp = ctx.enter_context(tc.tile_pool(name="otp", bufs=2))

        stqk_ps = ctx.enter_context(
            tc.tile_pool(name="stqk_ps", bufs=2, space="PSUM"))
        vt_ps = ctx.enter_context(
            tc.tile_pool(name="vt_ps", bufs=2, space="PSUM"))
        pv_ps = ctx.enter_context(
            tc.tile_pool(name="pv_ps", bufs=2, space="PSUM"))

        def load_x(g):
            x8 = xbp.tile([128, 2, PIX], F8, tag="x8", name=f"x8_{g}")
            nc.sync.dma_start(out=x8, in_=x_d[g])
            return x8

        x0_early = load_x(0)

        # ---- constants (issued after x(0) so the first conv input leads) ----
        wqk_b = consts.tile([128, 2, 64], F8, tag="wqk")
        nc.sync.dma_start(out=wqk_b, in_=wqk_d[:])
        wvt_b = consts.tile([128, 2, 256], F8, tag="wvt")
        for h in range(2):
            nc.sync.dma_start(out=wvt_b[:, h, :], in_=wvt_d[h])
        bqk_b = consts.tile([128, 1], FP32, tag="bqk")  # [bq;0;bq;0]
        nc.sync.dma_start(out=bqk_b, in_=bqk_d[:])

        def emit_a1(g, x8):

            # qk conv: psum[128,512] per 1024 px: rows 0:64 = [q;k](even 512
            # block), rows 64:128 = [q;k](odd block); Q bias via ACT evac
            qk2 = qk2p.tile([128, 2, 512], BF16, tag="qk", name=f"qk2_{g}")
            for gb in range(2):
                qps = stqk_ps.tile([128, 512], FP32, tag="st")
                sa = slice((2 * gb) * 512, (2 * gb + 1) * 512)
                sb = slice((2 * gb + 1) * 512, (2 * gb + 2) * 512)
                nc.tensor.matmul(qps[0:64, :], wqk_b[:, 0, :], x8[:, 0, sa],
                                 start=True, stop=False, skip_group_check=True)
                nc.tensor.matmul(qps[0:64, :], wqk_b[:, 1, :], x8[:, 1, sa],
                                 start=False, stop=True, skip_group_check=True)
                nc.tensor.matmul(qps[64:128, :], wqk_b[:, 0, :], x8[:, 0, sb],
                                 start=True, stop=False, skip_group_check=True)
                nc.tensor.matmul(qps[64:128, :], wqk_b[:, 1, :], x8[:, 1, sb],
                                 start=False, stop=True, skip_group_check=True)
                nc.vector.tensor_scalar_add(qk2[:, gb, :], qps,
                                            bqk_b[:, 0:1])

            # vT conv into vt[128, 16, 257] bf16 (col 256 = 1.0 carrier)
            vt_g = vtp.tile([128, 16, 257], BF16, tag="vt", name=f"vt_{g}")
            nc.gpsimd.memset(vt_g[:, :, 256:257], 1.0)
            vt_engine = [nc.scalar, nc.vector, nc.scalar, nc.vector,
                         nc.scalar, nc.vector, nc.scalar, nc.vector]
            for vg in range(8):
                vps = vt_ps.tile([128, 2, 256], FP32, tag="vtps")
                for j in range(2):
                    p0 = vg * 256 + j * 128
                    nc.tensor.matmul(vps[:, j, :], x8[:, 0, p0:p0 + 128],
                                     wvt_b[:, 0, :], start=True, stop=False)
                    nc.tensor.matmul(vps[:, j, :], x8[:, 1, p0:p0 + 128],
                                     wvt_b[:, 1, :], start=False, stop=True)
                eng = vt_engine[vg]
                dst = vt_g[:, 2 * vg:2 * vg + 2, 0:256]
                if eng is nc.scalar:
                    nc.scalar.activation(out=dst, in_=vps, func=AF.Copy)
                else:
                    eng.tensor_copy(out=dst, in_=vps)

            # gather q and k to partition base 0 (2 merged DMAs each):
            # pixel p = gb*1024 + lo*512 + c lives at qk2[lo*64 + {q:0:32,
            # k:32:64}, gb, c]
            q0 = qkg.tile([32, PIX], BF16, tag="q0", name=f"q0_{g}")
            k0 = qkg.tile([32, PIX], BF16, tag="k0", name=f"k0_{g}")
            for lo in range(2):
                src_q = qk2[lo * 64:lo * 64 + 32, :, :]
                src_k = qk2[lo * 64 + 32:lo * 64 + 64, :, :]
                for dst_t, src in ((q0, src_q), (k0, src_k)):
                    base = dst_t[:, :]
                    dst = bass.AP(tensor=base.tensor,
                                  offset=base.offset + lo * 512,
                                  ap=[[2048, 32], [1024, 2], [1, 512]])
                    nc.sync.dma_start(out=dst, in_=src)

            return qk2, vt_g, q0, k0

        def emit_a2(g, parts):
            qk2, vt_g, q0, k0 = parts
            # sT pair matmuls: pair pp = windows (2pp, 2pp+1) = 128 px;
            # diagonal 64x64 blocks valid, off-diagonal = cross-window
            # scores (finite, never read). 4 pairs per [128,512] psum.
            eT_g = etp.tile([128, 4, 512], BF16, tag="eT", name=f"eT_{g}")
            for sg in range(4):
                sps = stqk_ps.tile([128, 512], FP32, tag="st")
                for pl in range(4):
                    pp = sg * 4 + pl
                    e0 = slice(pp * 128, pp * 128 + 64)
                    e1 = slice(pp * 128 + 64, (pp + 1) * 128)
                    nc.tensor.matmul(sps[0:64, pl * 128:pl * 128 + 64],
                                     k0[:, e0], q0[:, e0],
                                     start=True, stop=True,
                                     skip_group_check=True)
                    nc.tensor.matmul(sps[64:128, pl * 128 + 64:(pl + 1) * 128],
                                     k0[:, e1], q0[:, e1],
                                     start=True, stop=True,
                                     skip_group_check=True)
                nc.scalar.activation(out=eT_g[:, sg, :], in_=sps, func=AF.Exp)
                # zero the cross-window blocks so PV can contract the full
                # 128-pixel pair in one K=128 matmul
                top = eT_g[0:64, sg, :]
                nc.gpsimd.memset(
                    bass.AP(tensor=top.tensor, offset=top.offset + 64,
                            ap=[[2048, 64], [128, 4], [1, 64]]), 0.0)
                bot = eT_g[64:128, sg, :]
                nc.gpsimd.memset(
                    bass.AP(tensor=bot.tensor, offset=bot.offset,
                            ap=[[2048, 64], [128, 4], [1, 64]]), 0.0)
            return eT_g, vt_g

        def emit_b(g, state):
            eT_g, vt_g = state
            oT_g = otp.tile([128, 16, 257], BF16, tag="oT", name=f"oT_{g}")
            for q2 in range(8):
                pv2 = pv_ps.tile([128, 2, 512], FP32, tag="pv")
                for pi in range(2):
                    p = q2 * 2 + pi
                    sg, ec = p // 4, (p % 4) * 128
                    nc.tensor.matmul(pv2[:, pi, 0:257],
                                     eT_g[:, sg, ec:ec + 128],
                                     vt_g[:, p, :], start=True, stop=True)
                dst = oT_g[:, 2 * q2:2 * q2 + 2, :]
                if q2 in (0, 3, 6):
                    nc.scalar.activation(out=dst, in_=pv2[:, :, 0:257],
                                         func=AF.Copy)
                else:
                    nc.vector.tensor_copy(out=dst, in_=pv2[:, :, 0:257])
                if q2 == 3:
                    nc.sync.dma_start(out=out_d[g, :, 0:8, :],
                                      in_=oT_g[:, 0:8, :])
            nc.sync.dma_start(out=out_d[g, :, 8:16, :], in_=oT_g[:, 8:16, :])

        prev = None
        xq = {0: x0_early, 1: load_x(1)}
        for g in range(G):
            parts = emit_a1(g, xq.pop(g))
            if prev is not None:
                emit_b(g - 1, prev)
            state = emit_a2(g, parts)
            if g + 2 < G:
                xq[g + 2] = load_x(g + 2)
            prev = state
        emit_b(G - 1, prev)

    _split_wide_waits(nc)
    return nc


_NC_CACHE = None


def _get_nc():
    global _NC_CACHE
    if _NC_CACHE is None:
        _NC_CACHE = build_nc()
    return _NC_CACHE


def _prep_inputs(x, Wq, bq, Wk, bk, Wv, bv, gamma):
    """Host-side: pad + window-major permute + shard x; pack weights."""
    xp = np.zeros((B, C, PH, PW), np.float32)
    xp[:, :, :H, :W] = x
    # window-major: [b, c, nh, nw, r, wc] -> [b, c, wrow, pix]
    xw = xp.reshape(B, C, NH, A, NW, A).transpose(0, 1, 2, 4, 3, 5)
    xw = np.ascontiguousarray(xw).reshape(B, C, NH, PIX)

    shards = []
    for core in range(NCORES):
        b, hr = core // 2, core % 2
        sh = xw[b, :, hr * G:(hr + 1) * G, :]            # [256, G, PIX]
        sh = sh.reshape(2, 128, G, PIX).transpose(2, 1, 0, 3)
        shards.append(np.ascontiguousarray(sh).astype(ml_dtypes.float8_e4m3))

    wqk = np.concatenate([Wq.T, Wk.T], axis=1)          # [256, 64]
    wqk = wqk.reshape(2, 128, 64).transpose(1, 0, 2)    # [c, pair, 64]
    wqk = np.ascontiguousarray(wqk).astype(ml_dtypes.float8_e4m3)
    wvt = Wv.T.reshape(2, 128, 256).astype(ml_dtypes.float8_e4m3)  # [in, out]
    bqk = np.zeros((128, 1), np.float32)
    bqk[0:32, 0] = bq
    bqk[64:96, 0] = bq

    in_maps = []
    for core in range(NCORES):
        in_maps.append({
            "x": shards[core],
            "wqk": wqk,
            "wvt": wvt,
            "bqk": bqk,
        })
    return in_maps


def _gather_output(results, x, bv, gamma):
    raw = np.stack([results[i]["out"].astype(np.float32)
                    for i in range(NCORES)])  # [8, G, 128, 16, C+1]
    attn = (raw[..., 0:C] / raw[..., C:C + 1]
            * np.float32(gamma[0]))  # normalize by rowsum carrier
    attn = attn.reshape(B, 2 * G, 128, 16, C).transpose(0, 1, 3, 2, 4)
    attn = attn.reshape(B, 2 * G, PIX, C).transpose(0, 3, 1, 2)  # [b,c,nh,pix]
    attn = attn.reshape(B, C, NH, NW, A, A).transpose(0, 1, 2, 4, 3, 5)
    attn = np.ascontiguousarray(attn).reshape(B, C, PH, PW)[:, :, :H, :W]
    gbv = (gamma.astype(np.float64)[0]
           * bv.astype(np.float64)).astype(np.float32)
    return x + gbv[None, :, None, None] + attn


def run(inputs, trace=False):
    nc = _get_nc()
    in_maps = _prep_inputs(**inputs)
    res = run_bass_kernel_spmd(nc, in_maps, core_ids=list(range(NCORES)),
                               trace=trace)
    out = _gather_output(res.results, np.asarray(inputs["x"], np.float32),
                         inputs["bv"], inputs["gamma"])
    return out, res


def kernel(**inputs):
    inputs = {k: np.asarray(v) for k, v in inputs.items()}
    out, _ = run(inputs)
    return out



# revision 18
# speedup vs baseline: 1.0586x; 1.0586x over previous
"""AreaSelfAttention Trainium2 kernel (8 NeuronCores, pure data parallel).

Reference computation (per full input):
  pad x [4,256,252,252] -> [4,256,256,256]; 1x1 convs q,k (256->32), v (256->256);
  8x8 windows; attn = softmax(q^T k) over j; out = v @ attn^T; unwindow;
  final = gamma * out + x  (crop back to 252x252).

Design (v2):
  - Shard over (batch, wrow-half): each core gets 16 "wrows" of 2048
    window-major pixels (32 windows). x ships ONCE per core as fp8 e4m3 in
    [128, 2(c-half), pix] layout, DMA'd in 4-wrow chunks (16KB/partition
    descriptors). Device output is UNNORMALIZED PV plus a rowsum carrier
    column, bf16 [wrow-pair, 128, 2, 16, 257], DMA'd per wrow-pair.
  - Host finishes: out = x + gamma*bv + gamma*(oT'/rowsum) in f32 (softmax
    affinity: attn(v+bv) = attn(v)+bv, so no bias/residual ships).
  - K-bias dropped (softmax-invariant); Q-bias rides the q evac.
  - All convs use fp8 DoubleRow matmuls (K=256 contracted in one
    instruction at 0.5 cycles/row): qk conv = 8 matmuls/wrow emitting q
    and k into separate psum banks at partition bases {0,32,64,96} (one
    512-px block per base), evac'd straight into the sT operand layout --
    no SBUF gather DMAs. vT conv = 16 DoubleRow matmuls/wrow (128-pixel
    pair-stationary), carrier column 256 memset to 1.0.
  - sT: one [32,128]x[32,128]->[128,128] matmul per window pair (16/wrow);
    the off-diagonal cross-window blocks are computed garbage, exp'd, then
    memset to 0 (where PV's K=128 pair contraction needs zeros).
  - PV: per pair K=128 N=257 bf16 matmul (eT stationary, vt moving with
    rowsum carrier), evac per 2 pairs.
  - Evacs are spread across ACT/DVE/GPSIMD per a static assignment table.
  - Software pipeline: x prefetched one 4-wrow chunk ahead; wrow g-1's PV
    emitted between wrow g's conv phase and sT phase.
"""

from contextlib import ExitStack

import numpy as np
import ml_dtypes

import bass_rust as br
import concourse.bass as bass
import concourse.tile as tile
from concourse import mybir
from concourse.bass_utils import run_bass_kernel_spmd

FP32 = mybir.dt.float32
BF16 = mybir.dt.bfloat16
F8 = mybir.dt.float8e4
AF = mybir.ActivationFunctionType
DR = mybir.MatmulPerfMode.DoubleRow

B, C, H, W = 4, 256, 252, 252
A = 8
PH = PW = 256
NH = NW = 32
CR = 32
NCORES = 8
G = 16          # wrows per core
PIX = 2048      # pixels per wrow (32 windows * 64)
XCH = 4         # wrows per x DMA chunk


def _split_wide_waits(nc, max_waits=1):
    """walrus on this toolchain rejects >1 sync wait per instruction; move
    excess waits onto preceding same-engine NoOps (equivalent semantics)."""
    n = 0
    for fn in nc.m.functions:
        for bb in fn.blocks:
            insts = list(bb.instructions)
            new, changed = [], False
            for inst in insts:
                si = inst.sync_info
                waits = list(si.on_wait) if si is not None else []
                if len(waits) > max_waits:
                    changed = True
                    chunks = [waits[i:i + max_waits]
                              for i in range(0, len(waits), max_waits)]
                    for ch in chunks[:-1]:
                        nop = br.InstNoOp(name=f"I-wsplit-{n}", ins=[], outs=[])
                        n += 1
                        nop.engine = inst.engine
                        nop.sync_info = br.SyncInfo(on_wait=ch, on_update=[])
                        new.append(nop)
                    inst.sync_info = br.SyncInfo(
                        on_wait=chunks[-1], on_update=list(si.on_update))
                new.append(inst)
            if changed:
                bb.instructions = new
    return n


def build_nc():
    nc = bass.Bass()
    x_d = nc.declare_dram_parameter("x", [G // XCH, 128, XCH * 2 * PIX], F8,
                                    isOutput=False)
    wqk_d = nc.declare_dram_parameter("wqk", [128, 2, 64], F8, isOutput=False)
    wvt_d = nc.declare_dram_parameter("wvt", [2, 128, 256], F8,
                                      isOutput=False)
    bq4_d = nc.declare_dram_parameter("bq4", [128, 1], FP32, isOutput=False)
    out_d = nc.declare_dram_parameter("out", [G // 2, 128, 2, 16, C + 1],
                                      BF16, isOutput=True)

    with tile.TileContext(nc) as tc, ExitStack() as ctx:
        consts = ctx.enter_context(tc.tile_pool(name="consts", bufs=1))
        xbp = ctx.enter_context(tc.tile_pool(name="xbp", bufs=2))
        qk0p = ctx.enter_context(tc.tile_pool(name="qk0p", bufs=2))
        etp = ctx.enter_context(tc.tile_pool(name="etp", bufs=2))
        vtp = ctx.enter_context(tc.tile_pool(name="vtp", bufs=2))
        otp = ctx.enter_context(tc.tile_pool(name="otp", bufs=2))

        stqk_ps = ctx.enter_context(
            tc.tile_pool(name="stqk_ps", bufs=2, space="PSUM"))
        vt_ps = ctx.enter_context(
            tc.tile_pool(name="vt_ps", bufs=2, space="PSUM"))
        pv_ps = ctx.enter_context(
            tc.tile_pool(name="pv_ps", bufs=2, space="PSUM"))

        def load_x(ch):
            xc = xbp.tile([128, XCH, 2, PIX], F8, tag="x8", name=f"x8_{ch}")
            nc.sync.dma_start(out=xc, in_=x_d[ch])
            return xc

        xc0 = load_x(0)

        # ---- constants (issued after x(0) so the first conv input leads) ----
        wqk_b = consts.tile([128, 2, 64], F8, tag="wqk")
        nc.sync.dma_start(out=wqk_b, in_=wqk_d[:])
        wvt_b = consts.tile([128, 2, 256], F8, tag="wvt")
        for h in range(2):
            nc.sync.dma_start(out=wvt_b[:, h, :], in_=wvt_d[h])
        bq4_b = consts.tile([128, 1], FP32, tag="bq4")  # bq tiled 4x
        nc.sync.dma_start(out=bq4_b, in_=bq4_d[:])

        # evac engine helpers -------------------------------------------------
        def evac(eng, dst, src):
            if eng == "a":
                nc.scalar.activation(out=dst, in_=src, func=AF.Copy)
            elif eng == "v":
                nc.vector.tensor_copy(out=dst, in_=src)
            else:
                nc.gpsimd.tensor_copy(out=dst, in_=src)

        def emit_a1(g, x8):
            # qk conv: q and k into separate psum banks; block b (512 px)
            # lands at partition base 32b. DoubleRow contracts both c-halves.
            qps = stqk_ps.tile([128, 512], FP32, tag="st")
            kps = stqk_ps.tile([128, 512], FP32, tag="st")
            # s3d3 (DoubleRow) matmuls only allow dst partition base 0/64,
            # so qk uses normal fp8 matmuls col-tiled at {0,32,64,96};
            # distinct col-strips run concurrently in the PE sub-arrays.
            for ps, c0 in ((qps, 0), (kps, 32)):
                for b4 in range(4):
                    sa = slice(b4 * 512, (b4 + 1) * 512)
                    for h in range(2):
                        nc.tensor.matmul(ps[32 * b4:32 * b4 + 32, :],
                                         wqk_b[:, h, c0:c0 + 32],
                                         x8[:, h, sa],
                                         start=(h == 0), stop=(h == 1),
                                         skip_group_check=True,
                                         tile_position=(0, 32 * b4))
            q0 = qk0p.tile([128, 512], BF16, tag="q0", name=f"q0_{g}")
            k0 = qk0p.tile([128, 512], BF16, tag="k0", name=f"k0_{g}")
            nc.scalar.add(q0, qps, bq4_b[:, 0:1])
            nc.vector.tensor_copy(out=k0, in_=kps)
            # evac split (GPSIMD cannot touch PSUM - SBUF memsets only):
            # ACT: 4 exp + q + 3 vt + 3 pv;  DVE: k + 5 vt + 5 pv

            # vT conv into vt[128, 16, 257] bf16 (col 256 = 1.0 carrier);
            # pair p = 128 px: stationary x block, DoubleRow over c-halves.
            vt_g = vtp.tile([128, 16, 257], BF16, tag="vt", name=f"vt_{g}")
            nc.gpsimd.memset(vt_g[:, :, 256:257], 1.0)
            vt_engine = ["a", "v", "a", "v", "v", "a", "v", "v"]
            for vg in range(8):
                vps = vt_ps.tile([128, 2, 256], FP32, tag="vtps")
                for j in range(2):
                    p0 = vg * 256 + j * 128
                    nc.tensor.matmul(vps[:, j, :], x8[:, :, p0:p0 + 128],
                                     wvt_b, perf_mode=DR,
                                     skip_group_check=True)
                evac(vt_engine[vg], vt_g[:, 2 * vg:2 * vg + 2, 0:256], vps)
            return q0, k0, vt_g

        def emit_a2(g, parts):
            q0, k0, vt_g = parts
            # sT pair matmuls: pair pp = windows (2pp, 2pp+1) = 128 px in
            # block sg at partition base 32sg; one [32,128]x[32,128] matmul
            # writes the full [128,128] block (off-diagonal = cross-window
            # garbage, zeroed after exp).
            eT_g = etp.tile([128, 4, 512], BF16, tag="eT", name=f"eT_{g}")
            for sg in range(4):
                sps = stqk_ps.tile([128, 512], FP32, tag="st")
                pb = slice(32 * sg, 32 * sg + 32)
                for pl in range(4):
                    cw = slice(pl * 128, (pl + 1) * 128)
                    nc.tensor.matmul(sps[:, cw], k0[pb, cw], q0[pb, cw],
                                     skip_group_check=True,
                                     tile_position=(32 * sg, 0))
                nc.scalar.activation(out=eT_g[:, sg, :], in_=sps, func=AF.Exp)
                top = eT_g[0:64, sg, :]
                nc.gpsimd.memset(
                    bass.AP(tensor=top.tensor, offset=top.offset + 64,
                            ap=[[2048, 64], [128, 4], [1, 64]]), 0.0)
                bot = eT_g[64:128, sg, :]
                nc.gpsimd.memset(
                    bass.AP(tensor=bot.tensor, offset=bot.offset,
                            ap=[[2048, 64], [128, 4], [1, 64]]), 0.0)
            return eT_g, vt_g

        def emit_b(g, state, oT_g):
            eT_g, vt_g = state
            pv_engine = ["a", "v", "a", "v", "v", "a", "v", "v"]
            for q2 in range(8):
                pv2 = pv_ps.tile([128, 2, 512], FP32, tag="pv")
                for pi in range(2):
                    p = q2 * 2 + pi
                    sg, ec = p // 4, (p % 4) * 128
                    nc.tensor.matmul(pv2[:, pi, 0:257],
                                     eT_g[:, sg, ec:ec + 128],
                                     vt_g[:, p, :], start=True, stop=True)
                dst = oT_g[:, g % 2, 2 * q2:2 * q2 + 2, :]
                evac(pv_engine[q2], dst, pv2[:, :, 0:257])
            if g % 2 == 1:
                nc.sync.dma_start(out=out_d[g // 2], in_=oT_g)

        prev = None
        oT_g = None
        xc = {0: xc0}
        for g in range(G):
            ch = g // XCH
            if g % XCH == 0 and ch + 1 < G // XCH:
                xc[ch + 1] = load_x(ch + 1)
            if g % 2 == 0:
                oT_g = otp.tile([128, 2, 16, C + 1], BF16, tag="oT",
                                name=f"oT_{g}")
            parts = emit_a1(g, xc[ch][:, g % XCH])
            if prev is not None:
                emit_b(g - 1, prev, oT_g if g % 2 == 1 else oT_prev)
            state = emit_a2(g, parts)
            prev = state
            oT_prev = oT_g
        emit_b(G - 1, prev, oT_g)

    _split_wide_waits(nc)
    return nc


_NC_CACHE = None


def _get_nc():
    global _NC_CACHE
    if _NC_CACHE is None:
        _NC_CACHE = build_nc()
    return _NC_CACHE


def _prep_inputs(x, Wq, bq, Wk, bk, Wv, bv, gamma):
    """Host-side: pad + window-major permute + shard x; pack weights."""
    xp = np.zeros((B, C, PH, PW), np.float32)
    xp[:, :, :H, :W] = x
    # window-major: [b, c, nh, nw, r, wc] -> [b, c, wrow, pix]
    xw = xp.reshape(B, C, NH, A, NW, A).transpose(0, 1, 2, 4, 3, 5)
    xw = np.ascontiguousarray(xw).reshape(B, C, NH, PIX)

    shards = []
    for core in range(NCORES):
        b, hr = core // 2, core % 2
        sh = xw[b, :, hr * G:(hr + 1) * G, :]            # [256, G, PIX]
        sh = sh.reshape(2, 128, G, PIX).transpose(2, 1, 0, 3)  # [G,128,2,PIX]
        # chunk 4 wrows per DMA: [G/4, 128, 4*2*PIX] per-partition contig
        sh = sh.reshape(G // XCH, XCH, 128, 2, PIX).transpose(0, 2, 1, 3, 4)
        sh = np.ascontiguousarray(sh).reshape(G // XCH, 128, XCH * 2 * PIX)
        shards.append(sh.astype(ml_dtypes.float8_e4m3))

    wqk = np.concatenate([Wq.T, Wk.T], axis=1)          # [256, 64]
    wqk = wqk.reshape(2, 128, 64).transpose(1, 0, 2)    # [c, pair, 64]
    wqk = np.ascontiguousarray(wqk).astype(ml_dtypes.float8_e4m3)
    wvt = Wv.T.reshape(2, 128, 256).astype(ml_dtypes.float8_e4m3)  # [in, out]
    bq4 = np.tile(bq, 4).reshape(128, 1).astype(np.float32)

    in_maps = []
    for core in range(NCORES):
        in_maps.append({
            "x": shards[core],
            "wqk": wqk,
            "wvt": wvt,
            "bq4": bq4,
        })
    return in_maps


def _gather_output(results, x, bv, gamma):
    raw = np.stack([results[i]["out"].astype(np.float32)
                    for i in range(NCORES)])  # [8, G/2, 128, 2, 16, C+1]
    raw = raw.transpose(0, 1, 3, 2, 4, 5).reshape(NCORES, G, 128, 16, C + 1)
    attn = (raw[..., 0:C] / raw[..., C:C + 1]
            * np.float32(gamma[0]))  # normalize by rowsum carrier
    attn = attn.reshape(B, 2 * G, 128, 16, C).transpose(0, 1, 3, 2, 4)
    attn = attn.reshape(B, 2 * G, PIX, C).transpose(0, 3, 1, 2)  # [b,c,nh,pix]
    attn = attn.reshape(B, C, NH, NW, A, A).transpose(0, 1, 2, 4, 3, 5)
    attn = np.ascontiguousarray(attn).reshape(B, C, PH, PW)[:, :, :H, :W]
    gbv = (gamma.astype(np.float64)[0]
           * bv.astype(np.float64)).astype(np.float32)
    return x + gbv[None, :, None, None] + attn


def run(inputs, trace=False):
    nc = _get_nc()
    in_maps = _prep_inputs(**inputs)
    res = run_bass_kernel_spmd(nc, in_maps, core_ids=list(range(NCORES)),
                               trace=trace)
    out = _gather_output(res.results, np.asarray(inputs["x"], np.float32),
                        inputs["bv"], inputs["gamma"])
    return out, res


def kernel(**inputs):
    inputs = {k: np.asarray(v) for k, v in inputs.items()}
    out, _ = run(inputs)
    return out


# revision 38
# speedup vs baseline: 1.1362x; 1.0733x over previous
"""AreaSelfAttention Trainium2 kernel (8 NeuronCores, pure data parallel).

Reference computation (per full input):
  pad x [4,256,252,252] -> [4,256,256,256]; 1x1 convs q,k (256->32), v (256->256);
  8x8 windows; attn = softmax(q^T k) over j; out = v @ attn^T; unwindow;
  final = gamma * out + x  (crop back to 252x252).

Design (v2):
  - Shard over (batch, wrow-half): each core gets 16 "wrows" of 2048
    window-major pixels (32 windows). x ships ONCE per core as fp8 e4m3 in
    [128, 2(c-half), pix] layout, DMA'd in 4-wrow chunks (16KB/partition
    descriptors). Device output is UNNORMALIZED PV plus a rowsum carrier
    column, bf16 [wrow-pair, 128, 2, 16, 257], DMA'd per wrow-pair.
  - Host finishes: out = x + gamma*bv + gamma*(oT'/rowsum) in f32 (softmax
    affinity: attn(v+bv) = attn(v)+bv, so no bias/residual ships).
  - K-bias dropped (softmax-invariant); Q-bias rides the q evac.
  - All convs use fp8 DoubleRow matmuls (K=256 contracted in one
    instruction at 0.5 cycles/row): qk conv = 8 matmuls/wrow emitting q
    and k into separate psum banks at partition bases {0,32,64,96} (one
    512-px block per base), evac'd straight into the sT operand layout --
    no SBUF gather DMAs. vT conv = 16 DoubleRow matmuls/wrow (128-pixel
    pair-stationary), carrier column 256 memset to 1.0.
  - sT: one [32,128]x[32,128]->[128,128] matmul per window pair (16/wrow);
    the off-diagonal cross-window blocks are computed garbage, exp'd, then
    memset to 0 (where PV's K=128 pair contraction needs zeros).
  - PV: per pair K=128 N=257 bf16 matmul (eT stationary, vt moving with
    rowsum carrier), evac per 2 pairs.
  - Evacs are spread across ACT/DVE/GPSIMD per a static assignment table.
  - Software pipeline: x prefetched one 4-wrow chunk ahead; wrow g-1's PV
    emitted between wrow g's conv phase and sT phase.
"""

from contextlib import ExitStack

import numpy as np
import ml_dtypes

import bass_rust as br
import concourse.bass as bass
import concourse.tile as tile
from concourse import mybir
from concourse.bass_utils import run_bass_kernel_spmd

FP32 = mybir.dt.float32
BF16 = mybir.dt.bfloat16
F8 = mybir.dt.float8e4
AF = mybir.ActivationFunctionType
DR = mybir.MatmulPerfMode.DoubleRow

B, C, H, W = 4, 256, 252, 252
A = 8
PH = PW = 256
NH = NW = 32
CR = 32
NCORES = 8
G = 16          # wrows per core
PIX = 2048      # pixels per wrow (32 windows * 64)


def _split_wide_waits(nc, max_waits=1):
    """walrus on this toolchain rejects >1 sync wait per instruction; move
    excess waits onto preceding same-engine NoOps (equivalent semantics)."""
    n = 0
    for fn in nc.m.functions:
        for bb in fn.blocks:
            insts = list(bb.instructions)
            new, changed = [], False
            for inst in insts:
                si = inst.sync_info
                waits = list(si.on_wait) if si is not None else []
                if len(waits) > max_waits:
                    changed = True
                    chunks = [waits[i:i + max_waits]
                              for i in range(0, len(waits), max_waits)]
                    for ch in chunks[:-1]:
                        nop = br.InstNoOp(name=f"I-wsplit-{n}", ins=[], outs=[])
                        n += 1
                        nop.engine = inst.engine
                        nop.sync_info = br.SyncInfo(on_wait=ch, on_update=[])
                        new.append(nop)
                    inst.sync_info = br.SyncInfo(
                        on_wait=chunks[-1], on_update=list(si.on_update))
                new.append(inst)
            if changed:
                bb.instructions = new
    return n


def build_nc():
    nc = bass.Bass()
    x_d = nc.declare_dram_parameter("x", [G, 128, 2, PIX], F8,
                                    isOutput=False)
    wqk_d = nc.declare_dram_parameter("wqk", [128, 2, 64], F8, isOutput=False)
    wvt_d = nc.declare_dram_parameter("wvt", [2, 128, 256], F8,
                                      isOutput=False)
    bq4_d = nc.declare_dram_parameter("bq4", [128, 1], FP32, isOutput=False)
    out_d = nc.declare_dram_parameter("out", [G, 128, 16, C + 1], BF16,
                                      isOutput=True)

    with tile.TileContext(nc) as tc, ExitStack() as ctx:
        consts = ctx.enter_context(tc.tile_pool(name="consts", bufs=1))
        xbp = ctx.enter_context(tc.tile_pool(name="xbp", bufs=3))
        qk0p = ctx.enter_context(tc.tile_pool(name="qk0p", bufs=2))
        etp = ctx.enter_context(tc.tile_pool(name="etp", bufs=2))
        vtp = ctx.enter_context(tc.tile_pool(name="vtp", bufs=2))
        otp = ctx.enter_context(tc.tile_pool(name="otp", bufs=2))

        stqk_ps = ctx.enter_context(
            tc.tile_pool(name="stqk_ps", bufs=2, space="PSUM"))
        vt_ps = ctx.enter_context(
            tc.tile_pool(name="vt_ps", bufs=2, space="PSUM"))
        pv_ps = ctx.enter_context(
            tc.tile_pool(name="pv_ps", bufs=2, space="PSUM"))

        def load_x(g):
            x8 = xbp.tile([128, 2, PIX], F8, tag="x8", name=f"x8_{g}")
            nc.sync.dma_start(out=x8, in_=x_d[g])
            return x8

        x0_early = load_x(0)

        # ---- constants (issued after x(0) so the first conv input leads) ----
        wqk_b = consts.tile([128, 2, 64], F8, tag="wqk")
        nc.sync.dma_start(out=wqk_b, in_=wqk_d[:])
        wvt_b = consts.tile([128, 2, 256], F8, tag="wvt")
        for h in range(2):
            nc.sync.dma_start(out=wvt_b[:, h, :], in_=wvt_d[h])
        bq4_b = consts.tile([128, 1], FP32, tag="bq4")  # bq tiled 4x
        nc.sync.dma_start(out=bq4_b, in_=bq4_d[:])

        # evac engine helpers -------------------------------------------------
        def evac(eng, dst, src):
            if eng == "a":
                nc.scalar.activation(out=dst, in_=src, func=AF.Copy)
            elif eng == "v":
                nc.vector.tensor_copy(out=dst, in_=src)
            else:
                nc.gpsimd.tensor_copy(out=dst, in_=src)

        def emit_a1(g, x8):
            # qk conv: q and k into separate psum banks; block b (512 px)
            # lands at partition base 32b. DoubleRow contracts both c-halves.
            qps = stqk_ps.tile([128, 512], FP32, tag="st")
            kps = stqk_ps.tile([128, 512], FP32, tag="st")
            # qk conv: normal fp8 matmuls col-tiled at {0,32,64,96} (s3d3/
            # DoubleRow requires dst base 0, unusable here). Strip round-
            # robin order: the 4 col-strips run concurrently in distinct PE
            # sub-arrays; the c-half accumulation pairs serialize per strip.
            for ps, c0 in ((qps, 0), (kps, 32)):
                for h in range(2):
                    for b4 in range(4):
                        sa = slice(b4 * 512, (b4 + 1) * 512)
                        nc.tensor.matmul(ps[32 * b4:32 * b4 + 32, :],
                                         wqk_b[:, h, c0:c0 + 32],
                                         x8[:, h, sa],
                                         start=(h == 0), stop=(h == 1),
                                         skip_group_check=True,
                                         tile_position=(0, 32 * b4))
            q0 = qk0p.tile([128, 512], BF16, tag="q0", name=f"q0_{g}")
            k0 = qk0p.tile([128, 512], BF16, tag="k0", name=f"k0_{g}")
            nc.scalar.add(q0, qps, bq4_b[:, 0:1])
            nc.vector.tensor_copy(out=k0, in_=kps)
            # evac split (GPSIMD cannot touch PSUM - SBUF memsets only):
            # ACT: 4 exp + q + 3 vt + 3 pv;  DVE: k + 5 vt + 5 pv

            # vT conv into vt[128, 16, 257] bf16 (col 256 = 1.0 carrier);
            # pair p = 128 px: stationary x block, DoubleRow over c-halves.
            vt_g = vtp.tile([128, 16, 257], BF16, tag="vt", name=f"vt_{g}")
            nc.gpsimd.memset(vt_g[:, :, 256:257], 1.0)
            vt_engine = ["a", "v", "a", "v", "v", "a", "v", "v"]
            for vg in range(8):
                vps = vt_ps.tile([128, 2, 256], FP32, tag="vtps")
                for j in range(2):
                    p0 = vg * 256 + j * 128
                    nc.tensor.matmul(vps[:, j, :], x8[:, :, p0:p0 + 128],
                                     wvt_b, perf_mode=DR,
                                     skip_group_check=True)
                evac(vt_engine[vg], vt_g[:, 2 * vg:2 * vg + 2, 0:256], vps)
            return q0, k0, vt_g

        def emit_a2(g, parts):
            q0, k0, vt_g = parts
            # sT pair matmuls: pair pp = windows (2pp, 2pp+1) = 128 px in
            # block sg at partition base 32sg; one [32,128]x[32,128] matmul
            # writes the full [128,128] block (off-diagonal = cross-window
            # garbage, zeroed after exp).
            eT_g = etp.tile([128, 4, 512], BF16, tag="eT", name=f"eT_{g}")
            for sg in range(4):
                sps = stqk_ps.tile([128, 512], FP32, tag="st")
                pb = slice(32 * sg, 32 * sg + 32)
                for pl in range(4):
                    cw = slice(pl * 128, (pl + 1) * 128)
                    nc.tensor.matmul(sps[:, cw], k0[pb, cw], q0[pb, cw],
                                     skip_group_check=True,
                                     tile_position=(32 * sg, 0))
                nc.scalar.activation(out=eT_g[:, sg, :], in_=sps, func=AF.Exp)
                top = eT_g[0:64, sg, :]
                nc.gpsimd.memset(
                    bass.AP(tensor=top.tensor, offset=top.offset + 64,
                            ap=[[2048, 64], [128, 4], [1, 64]]), 0.0)
                bot = eT_g[64:128, sg, :]
                nc.gpsimd.memset(
                    bass.AP(tensor=bot.tensor, offset=bot.offset,
                            ap=[[2048, 64], [128, 4], [1, 64]]), 0.0)
            return eT_g, vt_g

        def emit_b(g, state, oT_g):
            eT_g, vt_g = state
            pv_engine = ["a", "v", "a", "v", "v", "a", "v", "v"]
            for q2 in range(8):
                pv2 = pv_ps.tile([128, 2, 512], FP32, tag="pv")
                for pi in range(2):
                    p = q2 * 2 + pi
                    sg, ec = p // 4, (p % 4) * 128
                    nc.tensor.matmul(pv2[:, pi, 0:257],
                                     eT_g[:, sg, ec:ec + 128],
                                     vt_g[:, p, :], start=True, stop=True)
                dst = oT_g[:, 2 * q2:2 * q2 + 2, :]
                evac(pv_engine[q2], dst, pv2[:, :, 0:257])
            nc.sync.dma_start(out=out_d[g], in_=oT_g)

        prev = None
        xq = {0: x0_early, 1: load_x(1)}
        for g in range(G):
            parts = emit_a1(g, xq.pop(g))
            if prev is not None:
                oT_p = otp.tile([128, 16, C + 1], BF16, tag="oT",
                                name=f"oT_{g - 1}")
                emit_b(g - 1, prev, oT_p)
            state = emit_a2(g, parts)
            if g + 2 < G:
                xq[g + 2] = load_x(g + 2)
            prev = state
        oT_p = otp.tile([128, 16, C + 1], BF16, tag="oT", name=f"oT_{G - 1}")
        emit_b(G - 1, prev, oT_p)

    _split_wide_waits(nc)
    return nc


_NC_CACHE = None


def _get_nc():
    global _NC_CACHE
    if _NC_CACHE is None:
        _NC_CACHE = build_nc()
    return _NC_CACHE


def _prep_inputs(x, Wq, bq, Wk, bk, Wv, bv, gamma):
    """Host-side: pad + window-major permute + shard x; pack weights."""
    xp = np.zeros((B, C, PH, PW), np.float32)
    xp[:, :, :H, :W] = x
    # window-major: [b, c, nh, nw, r, wc] -> [b, c, wrow, pix]
    xw = xp.reshape(B, C, NH, A, NW, A).transpose(0, 1, 2, 4, 3, 5)
    xw = np.ascontiguousarray(xw).reshape(B, C, NH, PIX)

    shards = []
    for core in range(NCORES):
        b, hr = core // 2, core % 2
        sh = xw[b, :, hr * G:(hr + 1) * G, :]            # [256, G, PIX]
        sh = sh.reshape(2, 128, G, PIX).transpose(2, 1, 0, 3)  # [G,128,2,PIX]
        shards.append(np.ascontiguousarray(sh).astype(ml_dtypes.float8_e4m3))

    wqk = np.concatenate([Wq.T, Wk.T], axis=1)          # [256, 64]
    wqk = wqk.reshape(2, 128, 64).transpose(1, 0, 2)    # [c, pair, 64]
    wqk = np.ascontiguousarray(wqk).astype(ml_dtypes.float8_e4m3)
    wvt = Wv.T.reshape(2, 128, 256).astype(ml_dtypes.float8_e4m3)  # [in, out]
    bq4 = np.tile(bq, 4).reshape(128, 1).astype(np.float32)

    in_maps = []
    for core in range(NCORES):
        in_maps.append({
            "x": shards[core],
            "wqk": wqk,
            "wvt": wvt,
            "bq4": bq4,
        })
    return in_maps


def _gather_output(results, x, bv, gamma):
    raw = np.stack([results[i]["out"].astype(np.float32)
                    for i in range(NCORES)])  # [8, G, 128, 16, C+1]
    attn = (raw[..., 0:C] / raw[..., C:C + 1]
            * np.float32(gamma[0]))  # normalize by rowsum carrier
    attn = attn.reshape(B, 2 * G, 128, 16, C).transpose(0, 1, 3, 2, 4)
    attn = attn.reshape(B, 2 * G, PIX, C).transpose(0, 3, 1, 2)  # [b,c,nh,pix]
    attn = attn.reshape(B, C, NH, NW, A, A).transpose(0, 1, 2, 4, 3, 5)
    attn = np.ascontiguousarray(attn).reshape(B, C, PH, PW)[:, :, :H, :W]
    gbv = (gamma.astype(np.float64)[0]
           * bv.astype(np.float64)).astype(np.float32)
    return x + gbv[None, :, None, None] + attn


def run(inputs, trace=False):
    nc = _get_nc()
    in_maps = _prep_inputs(**inputs)
    res = run_bass_kernel_spmd(nc, in_maps, core_ids=list(range(NCORES)),
                               trace=trace)
    out = _gather_output(res.results, np.asarray(inputs["x"], np.float32),
                        inputs["bv"], inputs["gamma"])
    return out, res


def kernel(**inputs):
    inputs = {k: np.asarray(v) for k, v in inputs.items()}
    out, _ = run(inputs)
    return out


# revision 40
# speedup vs baseline: 1.2968x; 1.1414x over previous
"""AreaSelfAttention Trainium2 kernel (8 NeuronCores, pure data parallel).

Reference computation (per full input):
  pad x [4,256,252,252] -> [4,256,256,256]; 1x1 convs q,k (256->32), v (256->256);
  8x8 windows; attn = softmax(q^T k) over j; out = v @ attn^T; unwindow;
  final = gamma * out + x  (crop back to 252x252).

Design (v2):
  - Shard over (batch, wrow-half): each core gets 16 "wrows" of 2048
    window-major pixels (32 windows). x ships ONCE per core as fp8 e4m3 in
    [128, 2(c-half), pix] layout, DMA'd in 4-wrow chunks (16KB/partition
    descriptors). Device output is UNNORMALIZED PV plus a rowsum carrier
    column, bf16 [wrow-pair, 128, 2, 16, 257], DMA'd per wrow-pair.
  - Host finishes: out = x + gamma*bv + gamma*(oT'/rowsum) in f32 (softmax
    affinity: attn(v+bv) = attn(v)+bv, so no bias/residual ships).
  - K-bias dropped (softmax-invariant); Q-bias rides the q evac.
  - All convs use fp8 DoubleRow matmuls (K=256 contracted in one
    instruction at 0.5 cycles/row): qk conv = 8 matmuls/wrow emitting q
    and k into separate psum banks at partition bases {0,32,64,96} (one
    512-px block per base), evac'd straight into the sT operand layout --
    no SBUF gather DMAs. vT conv = 16 DoubleRow matmuls/wrow (128-pixel
    pair-stationary), carrier column 256 memset to 1.0.
  - sT: one [32,128]x[32,128]->[128,128] matmul per window pair (16/wrow);
    the off-diagonal cross-window blocks are computed garbage, exp'd, then
    memset to 0 (where PV's K=128 pair contraction needs zeros).
  - PV: per pair K=128 N=257 bf16 matmul (eT stationary, vt moving with
    rowsum carrier), evac per 2 pairs.
  - Evacs are spread across ACT/DVE/GPSIMD per a static assignment table.
  - Software pipeline: x prefetched one 4-wrow chunk ahead; wrow g-1's PV
    emitted between wrow g's conv phase and sT phase.
"""

from contextlib import ExitStack

import numpy as np
import ml_dtypes

import bass_rust as br
import concourse.bass as bass
import concourse.tile as tile
from concourse import mybir
from concourse.bass_utils import run_bass_kernel_spmd

FP32 = mybir.dt.float32
BF16 = mybir.dt.bfloat16
F8 = mybir.dt.float8e4
AF = mybir.ActivationFunctionType
DR = mybir.MatmulPerfMode.DoubleRow

B, C, H, W = 4, 256, 252, 252
A = 8
PH = PW = 256
NH = NW = 32
CR = 32
NCORES = 8
G = 16          # wrows per core
PIX = 2048      # pixels per wrow (32 windows * 64)


def _split_wide_waits(nc, max_waits=1):
    """walrus on this toolchain rejects >1 sync wait per instruction; move
    excess waits onto preceding same-engine NoOps (equivalent semantics)."""
    n = 0
    for fn in nc.m.functions:
        for bb in fn.blocks:
            insts = list(bb.instructions)
            new, changed = [], False
            for inst in insts:
                si = inst.sync_info
                waits = list(si.on_wait) if si is not None else []
                if len(waits) > max_waits:
                    changed = True
                    chunks = [waits[i:i + max_waits]
                              for i in range(0, len(waits), max_waits)]
                    for ch in chunks[:-1]:
                        nop = br.InstNoOp(name=f"I-wsplit-{n}", ins=[], outs=[])
                        n += 1
                        nop.engine = inst.engine
                        nop.sync_info = br.SyncInfo(on_wait=ch, on_update=[])
                        new.append(nop)
                    inst.sync_info = br.SyncInfo(
                        on_wait=chunks[-1], on_update=list(si.on_update))
                new.append(inst)
            if changed:
                bb.instructions = new
    return n


def build_nc():
    nc = bass.Bass()
    x_d = nc.declare_dram_parameter("x", [G, 128, 2, PIX], F8,
                                    isOutput=False)
    wqk_d = nc.declare_dram_parameter("wqk", [128, 2, 64], F8, isOutput=False)
    wvt_d = nc.declare_dram_parameter("wvt", [2, 128, 256], F8,
                                      isOutput=False)
    bq4_d = nc.declare_dram_parameter("bq4", [128, 1], FP32, isOutput=False)
    out_d = nc.declare_dram_parameter("out", [G, 128, 16, C + 1], BF16,
                                      isOutput=True)

    with tile.TileContext(nc) as tc, ExitStack() as ctx:
        consts = ctx.enter_context(tc.tile_pool(name="consts", bufs=1))
        xbp = ctx.enter_context(tc.tile_pool(name="xbp", bufs=3))
        qk0p = ctx.enter_context(tc.tile_pool(name="qk0p", bufs=2))
        etp = ctx.enter_context(tc.tile_pool(name="etp", bufs=2))
        vtp = ctx.enter_context(tc.tile_pool(name="vtp", bufs=2))
        otp = ctx.enter_context(tc.tile_pool(name="otp", bufs=2))

        stqk_ps = ctx.enter_context(
            tc.tile_pool(name="stqk_ps", bufs=2, space="PSUM"))
        vt_ps = ctx.enter_context(
            tc.tile_pool(name="vt_ps", bufs=2, space="PSUM"))
        pv_ps = ctx.enter_context(
            tc.tile_pool(name="pv_ps", bufs=2, space="PSUM"))

        def load_x(g):
            x8 = xbp.tile([128, 2, PIX], F8, tag="x8", name=f"x8_{g}")
            nc.sync.dma_start(out=x8, in_=x_d[g])
            return x8

        x0_early = load_x(0)

        # ---- constants (x(0) leads; wqk next so the first qk matmul can
        # start as soon as wrow 0 lands; x(1) only after the consts) ----
        wqk_b = consts.tile([128, 2, 64], F8, tag="wqk")
        nc.sync.dma_start(out=wqk_b, in_=wqk_d[:])
        bq4_b = consts.tile([128, 1], FP32, tag="bq4")  # bq tiled 4x
        nc.sync.dma_start(out=bq4_b, in_=bq4_d[:])
        wvt_b = consts.tile([128, 2, 256], F8, tag="wvt")
        for h in range(2):
            nc.sync.dma_start(out=wvt_b[:, h, :], in_=wvt_d[h])

        # evac engine helpers -------------------------------------------------
        def evac(eng, dst, src):
            if eng == "a":
                nc.scalar.activation(out=dst, in_=src, func=AF.Copy)
            elif eng == "v":
                nc.vector.tensor_copy(out=dst, in_=src)
            else:
                nc.gpsimd.tensor_copy(out=dst, in_=src)

        # evac engine tables: ACT: q + 4 exp + 3 vt + 3 pv;
        #                     DVE: k + 5 vt + 5 pv
        vt_engine = ["a", "v", "a", "v", "v", "a", "v", "v"]
        pv_engine = ["v", "a", "v", "a", "v", "a", "v", "v"]

        def emit_pv_group(gp, q2, state, oT_p):
            eT_p, vt_p = state
            pv2 = pv_ps.tile([128, 2, 512], FP32, tag="pv")
            for pi in range(2):
                p = q2 * 2 + pi
                sg, ec = p // 4, (p % 4) * 128
                nc.tensor.matmul(pv2[:, pi, 0:257],
                                 eT_p[:, sg, ec:ec + 128],
                                 vt_p[:, p, :], start=True, stop=True)
            dst = oT_p[:, 2 * q2:2 * q2 + 2, :]
            evac(pv_engine[q2], dst, pv2[:, :, 0:257])
            if q2 == 3:
                nc.sync.dma_start(out=out_d[gp, :, 0:8, :], in_=oT_p[:, 0:8, :])
            elif q2 == 7:
                nc.sync.dma_start(out=out_d[gp, :, 8:16, :],
                                  in_=oT_p[:, 8:16, :])

        def emit_wrow(g, x8, state):
            """One wrow's conv/score work, interleaved with wrow g-1's PV."""
            # qk conv: normal fp8 matmuls col-tiled at {0,32,64,96} (s3d3/
            # DoubleRow demands dst base 0, unusable here); the 4 strips
            # run concurrently in distinct PE sub-arrays, the c-half
            # accumulation pairs serialize per strip.
            qps = stqk_ps.tile([128, 512], FP32, tag="st")
            kps = stqk_ps.tile([128, 512], FP32, tag="st")
            for ps, c0 in ((qps, 0), (kps, 32)):
                for h in range(2):
                    for b4 in range(4):
                        sa = slice(b4 * 512, (b4 + 1) * 512)
                        nc.tensor.matmul(ps[32 * b4:32 * b4 + 32, :],
                                         wqk_b[:, h, c0:c0 + 32],
                                         x8[:, h, sa],
                                         start=(h == 0), stop=(h == 1),
                                         skip_group_check=True,
                                         tile_position=(0, 32 * b4))
            q0 = qk0p.tile([128, 512], BF16, tag="q0", name=f"q0_{g}")
            k0 = qk0p.tile([128, 512], BF16, tag="k0", name=f"k0_{g}")
            nc.scalar.add(q0, qps, bq4_b[:, 0:1])
            nc.vector.tensor_copy(out=k0, in_=kps)

            oT_p = None
            if state is not None:
                oT_p = otp.tile([128, 16, C + 1], BF16, tag="oT",
                                name=f"oT_{g - 1}")

            # vT conv into vt[128, 16, 257] bf16 (col 256 = 1.0 carrier);
            # pair p = 128 px: x-block stationary, DoubleRow over c-halves.
            # Interleave: vt group i, then PV group i of wrow g-1, then
            # (for i>=4) sT group i-4 -- smooths the evac-engine streams.
            vt_g = vtp.tile([128, 16, 257], BF16, tag="vt", name=f"vt_{g}")
            nc.gpsimd.memset(vt_g[:, :, 256:257], 1.0)
            eT_g = etp.tile([128, 4, 512], BF16, tag="eT", name=f"eT_{g}")
            for i in range(8):
                vps = vt_ps.tile([128, 2, 256], FP32, tag="vtps")
                for j in range(2):
                    p0 = i * 256 + j * 128
                    nc.tensor.matmul(vps[:, j, :], x8[:, :, p0:p0 + 128],
                                     wvt_b, perf_mode=DR,
                                     skip_group_check=True)
                evac(vt_engine[i], vt_g[:, 2 * i:2 * i + 2, 0:256], vps)
                if state is not None:
                    emit_pv_group(g - 1, i, state, oT_p)
                if i >= 4:
                    # sT pair matmuls for block sg = i-4 at partition base
                    # 32sg; one [32,128]x[32,128] matmul per pair writes
                    # the full [128,128] block (off-diagonal cross-window
                    # garbage exp'd then zeroed).
                    sg = i - 4
                    sps = stqk_ps.tile([128, 512], FP32, tag="st")
                    pb = slice(32 * sg, 32 * sg + 32)
                    for pl in range(4):
                        cw = slice(pl * 128, (pl + 1) * 128)
                        nc.tensor.matmul(sps[:, cw], k0[pb, cw], q0[pb, cw],
                                         skip_group_check=True,
                                         tile_position=(32 * sg, 0))
                    nc.scalar.activation(out=eT_g[:, sg, :], in_=sps,
                                         func=AF.Exp)
                    top = eT_g[0:64, sg, :]
                    nc.gpsimd.memset(
                        bass.AP(tensor=top.tensor, offset=top.offset + 64,
                                ap=[[2048, 64], [128, 4], [1, 64]]), 0.0)
                    bot = eT_g[64:128, sg, :]
                    nc.gpsimd.memset(
                        bass.AP(tensor=bot.tensor, offset=bot.offset,
                                ap=[[2048, 64], [128, 4], [1, 64]]), 0.0)
            return eT_g, vt_g

        prev = None
        xq = {0: x0_early, 1: load_x(1)}
        for g in range(G):
            state = emit_wrow(g, xq.pop(g), prev)
            if g + 2 < G:
                xq[g + 2] = load_x(g + 2)
            prev = state
        # flush wrow G-1's PV
        oT_p = otp.tile([128, 16, C + 1], BF16, tag="oT", name=f"oT_{G - 1}")
        for q2 in range(8):
            emit_pv_group(G - 1, q2, prev, oT_p)

    _split_wide_waits(nc)
    return nc


_NC_CACHE = None


def _get_nc():
    global _NC_CACHE
    if _NC_CACHE is None:
        _NC_CACHE = build_nc()
    return _NC_CACHE


def _prep_inputs(x, Wq, bq, Wk, bk, Wv, bv, gamma):
    """Host-side: pad + window-major permute + shard x; pack weights."""
    xp = np.zeros((B, C, PH, PW), np.float32)
    xp[:, :, :H, :W] = x
    # window-major: [b, c, nh, nw, r, wc] -> [b, c, wrow, pix]
    xw = xp.reshape(B, C, NH, A, NW, A).transpose(0, 1, 2, 4, 3, 5)
    xw = np.ascontiguousarray(xw).reshape(B, C, NH, PIX)

    shards = []
    for core in range(NCORES):
        b, hr = core // 2, core % 2
        sh = xw[b, :, hr * G:(hr + 1) * G, :]            # [256, G, PIX]
        sh = sh.reshape(2, 128, G, PIX).transpose(2, 1, 0, 3)  # [G,128,2,PIX]
        shards.append(np.ascontiguousarray(sh).astype(ml_dtypes.float8_e4m3))

    wqk = np.concatenate([Wq.T, Wk.T], axis=1)          # [256, 64]
    wqk = wqk.reshape(2, 128, 64).transpose(1, 0, 2)    # [c, pair, 64]
    wqk = np.ascontiguousarray(wqk).astype(ml_dtypes.float8_e4m3)
    wvt = Wv.T.reshape(2, 128, 256).astype(ml_dtypes.float8_e4m3)  # [in, out]
    bq4 = np.tile(bq, 4).reshape(128, 1).astype(np.float32)

    in_maps = []
    for core in range(NCORES):
        in_maps.append({
            "x": shards[core],
            "wqk": wqk,
            "wvt": wvt,
            "bq4": bq4,
        })
    return in_maps


def _gather_output(results, x, bv, gamma):
    raw = np.stack([results[i]["out"].astype(np.float32)
                    for i in range(NCORES)])  # [8, G, 128, 16, C+1]
    attn = (raw[..., 0:C] / raw[..., C:C + 1]
            * np.float32(gamma[0]))  # normalize by rowsum carrier
    attn = attn.reshape(B, 2 * G, 128, 16, C).transpose(0, 1, 3, 2, 4)
    attn = attn.reshape(B, 2 * G, PIX, C).transpose(0, 3, 1, 2)  # [b,c,nh,pix]
    attn = attn.reshape(B, C, NH, NW, A, A).transpose(0, 1, 2, 4, 3, 5)
    attn = np.ascontiguousarray(attn).reshape(B, C, PH, PW)[:, :, :H, :W]
    gbv = (gamma.astype(np.float64)[0]
           * bv.astype(np.float64)).astype(np.float32)
    return x + gbv[None, :, None, None] + attn


def run(inputs, trace=False):
    nc = _get_nc()
    in_maps = _prep_inputs(**inputs)
    res = run_bass_kernel_spmd(nc, in_maps, core_ids=list(range(NCORES)),
                               trace=trace)
    out = _gather_output(res.results, np.asarray(inputs["x"], np.float32),
                        inputs["bv"], inputs["gamma"])
    return out, res


def kernel(**inputs):
    inputs = {k: np.asarray(v) for k, v in inputs.items()}
    out, _ = run(inputs)
    return out
